# revision 12
# baseline (speedup 1.0000x reference)
"""Trainium2 Bass kernel for ContextQueryAttention (BiDAF-style), v2.

Math (per batch):
  S[i,j] = u[i] + v[j] + tri[i,j],  tri = (Q*wm) @ C^T (transposed view)
  S_row = softmax_j(S + NEG*qmask[j]);  S_col = softmax_i(S + NEG*cmask[i])
  A  = S_row @ Q
  Bt = S_row @ (S_col^T @ C)
  out = concat([C, A, C*A, C*Bt], -1)

v2 design (vs v1):
  - Factorized exponent: P1 = exp(tri + v - 15*qmask) only. u and cmask
    cancel in the row softmax; for the column softmax they enter through
    fm[i] = (1-cmask[i])*exp(u[i]) (exact-zero masking) applied to the
    T-GEMM moving operand CF = fm*C and the c1[j] = sum_i P1[j,i]*fm[i]
    weighted sums. This keeps every fp16 tensor in a healthy range.
  - All matmul operands fp16 (1 cycle/row on PE at any N; transposes 1.0
    cycles/row vs 1.5 for f32r).
  - Host pre-transposes C^T and (Q*wm)^T, precomputes u, v, fm, biasj.
    No C/Q transposes and no rank-1 score matmuls on device.
  - Device output is [A, C*A, C*Bt] fp16; host assembles the C column
    block (exact f32) and upcasts.
  - Data-parallel over batch: 32 batches -> 8 cores x 4 batches.
"""
import sys
sys.path.insert(0, "/opt/trn_rl_repo")

import numpy as np
from contextlib import ExitStack

from concourse import bass, bacc, mybir, tile, masks
from concourse.bass import AP
from concourse.bass_utils import run_bass_kernel_spmd

F32 = mybir.dt.float32
F16 = mybir.dt.float16
AF = mybir.ActivationFunctionType
OP = mybir.AluOpType

B, LC, LQ, D = 32, 1024, 256, 512
NCORES = 8
BPC = B // NCORES          # batches per core
MT, JT, KT = LC // 128, LQ // 128, D // 128   # 8, 2, 4
NEGB = -15.0               # qmask bias in log space; exp(-15) ~ 3e-7 (fp16-safe)

_CACHE = {}


def _build():
    nc = bacc.Bacc("TRN2", target_bir_lowering=False, debug=False)
    C_d = nc.dram_tensor("C16", [BPC, LC, D], F16, kind="ExternalInput")
    CT_d = nc.dram_tensor("CT16", [BPC, D, LC], F16, kind="ExternalInput")
    QWT_d = nc.dram_tensor("QWT16", [BPC, D, LQ], F16, kind="ExternalInput")
    Q_d = nc.dram_tensor("Q16", [BPC, LQ, D], F16, kind="ExternalInput")
    fm_d = nc.dram_tensor("fm16", [BPC, LC], F16, kind="ExternalInput")
    # cols = concat([fm (f32) col-layout, biasj col-layout]): [(8+2)*128] per batch
    cols_d = nc.dram_tensor("cols", [BPC, (MT + JT) * 128], F32, kind="ExternalInput")
    out_d = nc.dram_tensor("out", [BPC, LC, 3 * D], F16, kind="ExternalOutput")

    with tile.TileContext(nc) as tc, ExitStack() as ctx:
        const = ctx.enter_context(tc.tile_pool(name="const", bufs=1))
        big = ctx.enter_context(tc.tile_pool(name="big", bufs=2))
        mid = ctx.enter_context(tc.tile_pool(name="mid", bufs=2))
        sm = ctx.enter_context(tc.tile_pool(name="sm", bufs=2))
        pst = ctx.enter_context(tc.tile_pool(name="pst", bufs=2, space="PSUM"))
        ptr = ctx.enter_context(tc.tile_pool(name="ptr", bufs=1, space="PSUM"))
        ptt = ctx.enter_context(tc.tile_pool(name="ptt", bufs=2, space="PSUM"))
        pab = ctx.enter_context(tc.tile_pool(name="pab", bufs=3, space="PSUM"))

        # one-time identity (fp16) for PE transposes
        ident_f = const.tile([128, 128], F32)
        masks.make_identity(nc, ident_f[:])
        ident = const.tile([128, 128], F16)
        nc.vector.tensor_copy(ident[:], ident_f[:])

        for b in range(BPC):
            # ---------------- input DMAs ----------------
            # sync queue: score-path inputs first (QWT small, then CT per-k so
            # the first score matmul can start after ~256KB).
            QWT_sb = sm.tile([128, KT * LQ], F16, tag="QWT_sb")
            nc.sync.dma_start(QWT_sb[:].rearrange("p (k j) -> p k j", k=KT),
                              QWT_d.ap()[b].rearrange("(k p) j -> p k j", p=128))
            CT_sb = big.tile([128, KT * LC], F16, tag="CT_sb")     # [:, k*1024:] = CT k-tile
            for k in range(KT):
                nc.sync.dma_start(CT_sb[:, k * LC:(k + 1) * LC],
                                  CT_d.ap()[b, k * 128:(k + 1) * 128, :])
            # scalar queue: bias cols (tiny, needed by exp), Q, C
            cols = sm.tile([128, MT + JT], F32, tag="cols")
            nc.scalar.dma_start(cols[:], cols_d.ap()[b].rearrange("(t p) -> p t", p=128))
            fmcol = cols[:, 0:MT]
            bj_col = cols[:, MT:MT + JT]
            Q_sb = sm.tile([128, JT * D], F16, tag="Q_sb")
            nc.scalar.dma_start(Q_sb[:].rearrange("p (t d) -> p t d", t=JT),
                                Q_d.ap()[b].rearrange("(t p) d -> p t d", p=128))
            C_sb = big.tile([128, MT * D], F16, tag="C_sb")        # [:, it*512:] = C row-tile
            nc.scalar.dma_start(C_sb[:].rearrange("p (t d) -> p t d", t=MT),
                                C_d.ap()[b].rearrange("(t p) d -> p t d", p=128))
            # gpsimd queue: fm broadcast to all 128 partitions (stride-0)
            fmb = big.tile([128, LC], F16, tag="fmb")
            src = fm_d.ap()[b]
            nc.gpsimd.dma_start(fmb[:], AP(src.tensor, src.offset, [(0, 128), (1, LC)]))

            # ---------------- scores + exp:  PT1[j, i] ----------------
            PT1 = mid.tile([128, JT * LC], F16, tag="PT1")
            for jt in range(JT):
                for h in range(2):
                    ps_st = pst.tile([128, 512], F32, tag="pst")
                    for k in range(KT):
                        nc.tensor.matmul(
                            ps_st[:],
                            QWT_sb[:, k * LQ + jt * 128: k * LQ + (jt + 1) * 128],
                            CT_sb[:, k * LC + h * 512: k * LC + (h + 1) * 512],
                            start=(k == 0), stop=(k == KT - 1))
                    nc.scalar.activation(
                        PT1[:, jt * LC + h * 512: jt * LC + (h + 1) * 512],
                        ps_st[:], AF.Exp,
                        bias=bj_col[:, jt:jt + 1], scale=1.0)

            # ---------------- c1[j] = sum_i PT1[j,i] * fm[i] ----------------
            c1p = sm.tile([128, JT], F32, tag="c1p")
            scr = mid.tile([128, LC], F16, tag="scr")
            for jt in range(JT):
                nc.vector.scalar_tensor_tensor(
                    scr[:], PT1[:, jt * LC:(jt + 1) * LC], 1.0, fmb[:],
                    OP.mult, OP.mult, accum_out=c1p[:, jt:jt + 1])
            c1_rec = sm.tile([128, JT], F32, tag="c1_rec")
            nc.vector.reciprocal(c1_rec[:], c1p[:])

            # ---------------- transpose PT1 -> P1[i, j], row sums r ----------------
            P1 = mid.tile([128, MT * LQ], F16, tag="P1")
            r_acc = sm.tile([128, MT], F32, tag="r_acc")
            for itp in range(MT // 2):
                ps_tr = ptr.tile([128, 2 * LQ], F16, tag="ptr")
                for sub in range(2):
                    it = itp * 2 + sub
                    for jt in range(JT):
                        nc.tensor.transpose(
                            ps_tr[:, sub * LQ + jt * 128: sub * LQ + (jt + 1) * 128],
                            PT1[:, jt * LC + it * 128: jt * LC + (it + 1) * 128],
                            ident[:])
                    nc.vector.tensor_scalar(
                        P1[:, it * LQ:(it + 1) * LQ],
                        ps_tr[:, sub * LQ:(sub + 1) * LQ], 1.0, 0.0,
                        OP.mult, OP.add, accum_out=r_acc[:, it:it + 1])
            r_rec = sm.tile([128, MT], F32, tag="r_rec")
            nc.vector.reciprocal(r_rec[:], r_acc[:])

            # ---------------- CF = fm * C (gpsimd, SBUF-only) ----------------
            CF_sb = big.tile([128, MT * D], F16, tag="CF_sb")
            for it in range(MT):
                nc.gpsimd.tensor_scalar(
                    CF_sb[:, it * D:(it + 1) * D], C_sb[:, it * D:(it + 1) * D],
                    fmcol[:, it:it + 1], 0.0, OP.mult, OP.add)

            # ---------------- T[j, d] = (S_col^T C) = P1^T CF / c1 ----------------
            T16 = sm.tile([128, JT * D], F16, tag="T16")
            for jt in range(JT):
                ps_t = ptt.tile([128, 512], F32, tag="ptt")
                for it in range(MT):
                    nc.tensor.matmul(
                        ps_t[:],
                        P1[:, it * LQ + jt * 128: it * LQ + (jt + 1) * 128],
                        CF_sb[:, it * D:(it + 1) * D],
                        start=(it == 0), stop=(it == MT - 1))
                nc.vector.tensor_scalar_mul(
                    T16[:, jt * D:(jt + 1) * D], ps_t[:], c1_rec[:, jt:jt + 1])

            # ---------------- A, Bt, epilogue ----------------
            for itp in range(MT // 2):
                o16 = mid.tile([128, 2 * 1536], F16, tag="o16", bufs=3)
                for sub in range(2):
                    it = itp * 2 + sub
                    ps_a = pab.tile([128, 512], F32, tag="pab", name=f"psa{it % 3}")
                    ps_b = pab.tile([128, 512], F32, tag="pab", name=f"psb{it % 3}")
                    for jt in range(JT):
                        lhs = PT1[:, jt * LC + it * 128: jt * LC + (it + 1) * 128]
                        nc.tensor.matmul(ps_a[:], lhs, Q_sb[:, jt * D:(jt + 1) * D],
                                         start=(jt == 0), stop=(jt == JT - 1))
                        nc.tensor.matmul(ps_b[:], lhs, T16[:, jt * D:(jt + 1) * D],
                                         start=(jt == 0), stop=(jt == JT - 1))
                    o = o16[:, sub * 1536:(sub + 1) * 1536]
                    b16 = sm.tile([128, 512], F16, tag="b16", bufs=3)
                    nc.scalar.activation(o[:, 0:512], ps_a[:], AF.Copy,
                                         bias=0.0, scale=r_rec[:, it:it + 1])
                    nc.scalar.activation(b16[:], ps_b[:], AF.Copy,
                                         bias=0.0, scale=r_rec[:, it:it + 1])
                    nc.vector.tensor_tensor(o[:, 512:1024],
                                            C_sb[:, it * D:(it + 1) * D],
                                            o[:, 0:512], OP.mult)
                    nc.vector.tensor_tensor(o[:, 1024:1536],
                                            C_sb[:, it * D:(it + 1) * D],
                                            b16[:], OP.mult)
                nc.gpsimd.dma_start(
                    out_d.ap()[b, itp * 256:(itp + 1) * 256, :].rearrange(
                        "(t p) f -> p t f", p=128),
                    o16[:].rearrange("p (t f) -> p t f", t=2))
    nc.compile()
    return nc


def _get_nc():
    if "nc" not in _CACHE:
        _CACHE["nc"] = _build()
    return _CACHE["nc"]


def _prep(C, Q, W0, c_mask, q_mask):
    """Host-side precompute: fp16 operands, transposes, bias/scale vectors."""
    f16 = np.float16
    C = np.asarray(C, np.float32)
    Q = np.asarray(Q, np.float32)
    W0 = np.asarray(W0, np.float32)
    cm = np.asarray(c_mask, np.int32)
    qm = np.asarray(q_mask, np.int32)
    wc, wq, wm = W0[:D], W0[D:2 * D], W0[2 * D:]
    u = C @ wc                                     # [B, LC] f32
    v = Q @ wq                                     # [B, LQ] f32
    fm32 = (1.0 - cm).astype(np.float32) * np.exp(u)
    fm16 = fm32.astype(f16)
    biasj = (v + NEGB * qm).astype(np.float32)
    cols = np.concatenate([fm32, biasj], axis=1)   # [B, LC+LQ] f32
    C16 = C.astype(f16)
    CT16 = np.ascontiguousarray(C.transpose(0, 2, 1)).astype(f16)
    QWT16 = np.ascontiguousarray((Q * wm).transpose(0, 2, 1)).astype(f16)
    Q16 = Q.astype(f16)
    return dict(C16=C16, CT16=CT16, QWT16=QWT16, Q16=Q16,
                fm16=fm16, cols=cols)


def kernel(C, Q, W0, c_mask, q_mask):
    nc = _get_nc()
    C = np.ascontiguousarray(np.asarray(C, dtype=np.float32))
    pre = _prep(C, Q, W0, c_mask, q_mask)
    in_maps = []
    for c in range(NCORES):
        s = slice(c * BPC, (c + 1) * BPC)
        in_maps.append({k: np.ascontiguousarray(v[s]) for k, v in pre.items()})
    res = run_bass_kernel_spmd(nc, in_maps, core_ids=list(range(NCORES)))
    out = np.empty((B, LC, 4 * D), np.float32)
    out[:, :, 0:D] = C
    for c in range(NCORES):
        s = slice(c * BPC, (c + 1) * BPC)
        out[s, :, D:] = res.results[c]["out"].astype(np.float32)
    return out


if __name__ == "__main__":
    sys.path.insert(0, "/root/problem")
    import reference
    inputs = {k: np.asarray(v) for k, v in reference.setup_inputs().items()}
    expected = np.asarray(reference.reference(**inputs))
    actual = kernel(**inputs)
    err = np.abs(actual - expected)
    denom = np.abs(expected).max()
    print("max abs err:", err.max(), "rel:", err.max() / denom)


# revision 23
# speedup vs baseline: 1.0640x; 1.0640x over previous
"""Trainium2 Bass kernel for ContextQueryAttention (BiDAF-style), v3.

Math (per batch):
  S[i,j] = u[i] + v[j] + tri[i,j],  tri[i,j] = sum_d C[i,d]*wm[d]*Q[j,d]
  S_row = softmax_j(S + NEG*qmask[j]);  S_col = softmax_i(S + NEG*cmask[i])
  A  = S_row @ Q
  Bt = S_row @ (S_col^T @ C)
  out = concat([C, A, C*A, C*Bt], -1)

Design:
  - Factorized exponent: P1 = exp(tri + v - 15*qmask) only (u, cmask cancel
    in the row softmax). Column softmax gets them through fm[i] =
    (1-cmask[i])*exp(u[i]): T-GEMM moving operand CF = fm*C and weighted
    column sums c1[j] = sum_i P1[j,i]*fm[i].
  - All matmul operands fp16; psum f32. Zero PE transposes: PT1 -> P1 and
    CT -> C both use the DMA xbar transpose with 3D output descriptors.
  - Two batches interleaved phase-by-phase so cross-engine latencies hide
    behind the twin batch's PE work; all input DMAs prefetched up front.
  - Device output is [A, C*A, C*Bt] fp16; host assembles the C block.
  - Data-parallel over batch: 32 batches -> 8 cores x 4 batches.
"""
import sys
sys.path.insert(0, "/opt/trn_rl_repo")

import numpy as np
from contextlib import ExitStack

from concourse import bass, bacc, mybir, tile, masks
from concourse.bass import AP
from concourse.bass_utils import run_bass_kernel_spmd

F32 = mybir.dt.float32
F16 = mybir.dt.float16
AF = mybir.ActivationFunctionType
OP = mybir.AluOpType
AX = mybir.AxisListType

B, LC, LQ, D = 32, 1024, 256, 512
NCORES = 8
BPC = B // NCORES          # batches per core
MT, JT, KT = LC // 128, LQ // 128, D // 128   # 8, 2, 4
NEGB = -15.0               # qmask bias in log space; exp(-15) ~ 3e-7 (fp16-safe)

_CACHE = {}


def _build():
    nc = bacc.Bacc("TRN2", target_bir_lowering=False, debug=False)
    C_d = nc.dram_tensor("C16", [BPC, LC, D], F16, kind="ExternalInput")
    CT_d = nc.dram_tensor("CT16", [BPC, D, LC], F16, kind="ExternalInput")
    QWT_d = nc.dram_tensor("QWT16", [BPC, D, LQ], F16, kind="ExternalInput")
    Q_d = nc.dram_tensor("Q16", [BPC, LQ, D], F16, kind="ExternalInput")
    fm_d = nc.dram_tensor("fm16", [BPC, LC], F16, kind="ExternalInput")
    # per-partition cols: [BPC, 128, MT+JT] f32; [:, :, :MT]=fm, [:, :, MT:]=biasj
    cols_d = nc.dram_tensor("colsT", [BPC, 128, MT + JT], F32, kind="ExternalInput")
    out_d = nc.dram_tensor("out", [BPC, LC, 2 * D], F16, kind="ExternalOutput")

    with tile.TileContext(nc) as tc, ExitStack() as ctx:
        const = ctx.enter_context(tc.tile_pool(name="const", bufs=1))
        inp = ctx.enter_context(tc.tile_pool(name="inp", bufs=BPC))
        big = ctx.enter_context(tc.tile_pool(name="big", bufs=2))
        mid = ctx.enter_context(tc.tile_pool(name="mid", bufs=2))
        sm = ctx.enter_context(tc.tile_pool(name="sm", bufs=2))
        pst = ctx.enter_context(tc.tile_pool(name="pst", bufs=2, space="PSUM"))
        ptr = ctx.enter_context(tc.tile_pool(name="ptr", bufs=1, space="PSUM"))
        ptt = ctx.enter_context(tc.tile_pool(name="ptt", bufs=2, space="PSUM"))
        pab = ctx.enter_context(tc.tile_pool(name="pab", bufs=3, space="PSUM"))

        ident_f = const.tile([128, 128], F32)
        masks.make_identity(nc, ident_f[:])
        ident = const.tile([128, 128], F16)
        nc.vector.tensor_copy(ident[:], ident_f[:])

        st = {}  # per-batch tile state

        def phase_dma(b):
            s = st.setdefault(b, {})
            # sync queue: QWT (small) then CT per-k, then derive C = CT^T via xbar
            s["QWT"] = inp.tile([128, KT * LQ], F16, tag="QWT", name=f"QWT{b}")
            nc.sync.dma_start(s["QWT"][:].rearrange("p (k j) -> p k j", k=KT),
                              QWT_d.ap()[b].rearrange("(k p) j -> p k j", p=128))
            s["CT"] = inp.tile([128, KT * LC], F16, tag="CT", name=f"CT{b}")
            for k in range(KT):
                nc.sync.dma_start(s["CT"][:, k * LC:(k + 1) * LC],
                                  CT_d.ap()[b, k * 128:(k + 1) * 128, :])
            s["cols"] = inp.tile([128, MT + JT], F32, tag="cols", name=f"cols{b}")
            nc.sync.dma_start(s["cols"][:], cols_d.ap()[b])
            s["Q"] = inp.tile([128, JT * D], F16, tag="Q", name=f"Q{b}")
            nc.sync.dma_start(s["Q"][:].rearrange("p (t d) -> p t d", t=JT),
                              Q_d.ap()[b].rearrange("(t p) d -> p t d", p=128))
            # gpsimd queue: fm broadcast (stride-0) and C16
            s["fmb"] = inp.tile([128, LC], F16, tag="fmb", name=f"fmb{b}")
            src = fm_d.ap()[b]
            nc.gpsimd.dma_start(s["fmb"][:], AP(src.tensor, src.offset, [(0, 128), (1, LC)]))
            s["C"] = big.tile([128, MT * D], F16, tag="C", name=f"C{b % 2}")
            nc.gpsimd.dma_start(s["C"][:].rearrange("p (t d) -> p t d", t=MT),
                                C_d.ap()[b].rearrange("(t p) d -> p t d", p=128))

        def phase_scores(b):
            s = st[b]
            bj = s["cols"][:, MT:MT + JT]
            s["PT1"] = mid.tile([128, JT * LC], F16, tag="PT1", name=f"PT1{b % 2}")
            for jt in range(JT):
                for h in range(2):
                    ps_st = pst.tile([128, 512], F32, tag="pst")
                    for k in range(KT):
                        nc.tensor.matmul(
                            ps_st[:],
                            s["QWT"][:, k * LQ + jt * 128: k * LQ + (jt + 1) * 128],
                            s["CT"][:, k * LC + h * 512: k * LC + (h + 1) * 512],
                            start=(k == 0), stop=(k == KT - 1))
                    nc.scalar.activation(
                        s["PT1"][:, jt * LC + h * 512: jt * LC + (h + 1) * 512],
                        ps_st[:], AF.Exp, bias=bj[:, jt:jt + 1], scale=1.0)
            # CF = fm * C on gpsimd (early, feeds the T phase)
            fmcol = s["cols"][:, 0:MT]
            s["CF"] = big.tile([128, MT * D], F16, tag="CF", name=f"CF{b % 2}")
            for it in range(MT):
                nc.gpsimd.tensor_scalar(
                    s["CF"][:, it * D:(it + 1) * D], s["C"][:, it * D:(it + 1) * D],
                    fmcol[:, it:it + 1], 0.0, OP.mult, OP.add)

        def phase_transpose(b):
            s = st[b]
            # PT1 [j, (it i)] -> P1 [i, (it j)] via PE transposes; the psum->sbuf
            # copies carry the row-sum accumulation r for free
            s["P1"] = mid.tile([128, MT * LQ], F16, tag="P1", name=f"P1{b % 2}")
            s["r_acc"] = sm.tile([128, MT], F32, tag="r_acc", name=f"ra{b % 2}")
            for itp in range(MT // 2):
                ps_tr = ptr.tile([128, 2 * LQ], F16, tag="ptr")
                for sub in range(2):
                    it = itp * 2 + sub
                    for jt in range(JT):
                        nc.tensor.transpose(
                            ps_tr[:, sub * LQ + jt * 128: sub * LQ + (jt + 1) * 128],
                            s["PT1"][:, jt * LC + it * 128: jt * LC + (it + 1) * 128],
                            ident[:])
                    nc.vector.tensor_scalar(
                        s["P1"][:, it * LQ:(it + 1) * LQ],
                        ps_tr[:, sub * LQ:(sub + 1) * LQ], 1.0, 0.0,
                        OP.mult, OP.add, accum_out=s["r_acc"][:, it:it + 1])
            # c1[j] = sum_i PT1[j,i]*fm[i] on vector
            s["c1p"] = sm.tile([128, JT], F32, tag="c1p", name=f"c1p{b % 2}")
            scr = mid.tile([128, LC], F16, tag="scr", name=f"scr{b % 2}")
            for jt in range(JT):
                nc.vector.scalar_tensor_tensor(
                    scr[:], s["PT1"][:, jt * LC:(jt + 1) * LC], 1.0, s["fmb"][:],
                    OP.mult, OP.mult, accum_out=s["c1p"][:, jt:jt + 1])
            s["c1_rec"] = sm.tile([128, JT], F32, tag="c1r", name=f"c1r{b % 2}")
            nc.vector.reciprocal(s["c1_rec"][:], s["c1p"][:])
            s["r_rec"] = sm.tile([128, MT], F32, tag="r_rec", name=f"rr{b % 2}")
            nc.vector.reciprocal(s["r_rec"][:], s["r_acc"][:])

        def phase_T(b):
            s = st[b]
            s["T16"] = sm.tile([128, JT * D], F16, tag="T16", name=f"T16{b % 2}")
            for jt in range(JT):
                ps_t = ptt.tile([128, 512], F32, tag="ptt")
                for it in range(MT):
                    nc.tensor.matmul(
                        ps_t[:],
                        s["P1"][:, it * LQ + jt * 128: it * LQ + (jt + 1) * 128],
                        s["CF"][:, it * D:(it + 1) * D],
                        start=(it == 0), stop=(it == MT - 1))
                nc.vector.tensor_scalar_mul(
                    s["T16"][:, jt * D:(jt + 1) * D], ps_t[:], s["c1_rec"][:, jt:jt + 1])

        def phase_AB(b):
            s = st[b]
            for itp in range(MT // 4):
                o16 = mid.tile([128, 4 * 1024], F16, tag="o16", bufs=3)
                for sub in range(4):
                    it = itp * 4 + sub
                    ps_a = pab.tile([128, 512], F32, tag="pab", name=f"psa{it % 2}")
                    ps_b = pab.tile([128, 512], F32, tag="pab", name=f"psb{it % 2}")
                    for jt in range(JT):
                        lhs = s["PT1"][:, jt * LC + it * 128: jt * LC + (it + 1) * 128]
                        nc.tensor.matmul(ps_a[:], lhs, s["Q"][:, jt * D:(jt + 1) * D],
                                         start=(jt == 0), stop=(jt == JT - 1))
                        nc.tensor.matmul(ps_b[:], lhs, s["T16"][:, jt * D:(jt + 1) * D],
                                         start=(jt == 0), stop=(jt == JT - 1))
                    o = o16[:, sub * 1024:(sub + 1) * 1024]
                    nc.scalar.activation(o[:, 0:512], ps_a[:], AF.Copy,
                                         bias=0.0, scale=s["r_rec"][:, it:it + 1])
                    nc.vector.tensor_scalar_mul(o[:, 512:1024], ps_b[:],
                                                s["r_rec"][:, it:it + 1])
                nc.sync.dma_start(
                    out_d.ap()[b, itp * 512:(itp + 1) * 512, :].rearrange(
                        "(t p) f -> p t f", p=128),
                    o16[:].rearrange("p (t f) -> p t f", t=4))

        # two-batch software pipeline; next superbatch's inputs issued after
        # this superbatch's xbar transposes so they don't block them (FIFO)
        phase_dma(0)
        phase_dma(1)
        for sb in range(BPC // 2):
            b0, b1 = 2 * sb, 2 * sb + 1
            phase_scores(b0)
            phase_scores(b1)
            phase_transpose(b0)
            phase_transpose(b1)
            phase_T(b0)
            phase_T(b1)
            if b1 + 2 < BPC:
                phase_dma(b1 + 1)
                phase_dma(b1 + 2)
            phase_AB(b0)
            phase_AB(b1)
    nc.compile()
    return nc


def _get_nc():
    if "nc" not in _CACHE:
        _CACHE["nc"] = _build()
    return _CACHE["nc"]


def _prep(C, Q, W0, c_mask, q_mask):
    """Host-side precompute: fp16 operands, transposes, bias/scale vectors."""
    f16 = np.float16
    C = np.asarray(C, np.float32)
    Q = np.asarray(Q, np.float32)
    W0 = np.asarray(W0, np.float32)
    cm = np.asarray(c_mask, np.int32)
    qm = np.asarray(q_mask, np.int32)
    wc, wq, wm = W0[:D], W0[D:2 * D], W0[2 * D:]
    u = C @ wc                                     # [B, LC] f32
    v = Q @ wq                                     # [B, LQ] f32
    fm32 = (1.0 - cm).astype(np.float32) * np.exp(u)
    fm16 = fm32.astype(f16)
    biasj = (v + NEGB * qm).astype(np.float32)
    # per-partition col layout: [B, 128, MT+JT]
    colsT = np.concatenate([
        fm32.reshape(B, MT, 128).transpose(0, 2, 1),
        biasj.reshape(B, JT, 128).transpose(0, 2, 1)], axis=2)
    colsT = np.ascontiguousarray(colsT)
    C16 = C.astype(f16)
    CT16 = np.ascontiguousarray(C.transpose(0, 2, 1)).astype(f16)
    QWT16 = np.ascontiguousarray((Q * wm).transpose(0, 2, 1)).astype(f16)
    Q16 = Q.astype(f16)
    return dict(C16=C16, CT16=CT16, QWT16=QWT16, Q16=Q16, fm16=fm16, colsT=colsT)


def kernel(C, Q, W0, c_mask, q_mask):
    nc = _get_nc()
    C = np.ascontiguousarray(np.asarray(C, dtype=np.float32))
    pre = _prep(C, Q, W0, c_mask, q_mask)
    in_maps = []
    for c in range(NCORES):
        s = slice(c * BPC, (c + 1) * BPC)
        in_maps.append({k: np.ascontiguousarray(v[s]) for k, v in pre.items()})
    res = run_bass_kernel_spmd(nc, in_maps, core_ids=list(range(NCORES)))
    out = np.empty((B, LC, 4 * D), np.float32)
    out[:, :, 0:D] = C
    for c in range(NCORES):
        s = slice(c * BPC, (c + 1) * BPC)
        ab = res.results[c]["out"].astype(np.float32)
        A32 = ab[:, :, 0:D]
        B32 = ab[:, :, D:2 * D]
        out[s, :, D:2 * D] = A32
        out[s, :, 2 * D:3 * D] = C[s] * A32
        out[s, :, 3 * D:] = C[s] * B32
    return out


if __name__ == "__main__":
    sys.path.insert(0, "/root/problem")
    import reference
    inputs = {k: np.asarray(v) for k, v in reference.setup_inputs().items()}
    expected = np.asarray(reference.reference(**inputs))
    actual = kernel(**inputs)
    err = np.abs(actual - expected)
    denom = np.abs(expected).max()
    print("max abs err:", err.max(), "rel:", err.max() / denom)


# revision 28
# speedup vs baseline: 1.1504x; 1.0811x over previous
"""Trainium2 Bass kernel for ContextQueryAttention (BiDAF-style), v3.

Math (per batch):
  S[i,j] = u[i] + v[j] + tri[i,j],  tri[i,j] = sum_d C[i,d]*wm[d]*Q[j,d]
  S_row = softmax_j(S + NEG*qmask[j]);  S_col = softmax_i(S + NEG*cmask[i])
  A  = S_row @ Q
  Bt = S_row @ (S_col^T @ C)
  out = concat([C, A, C*A, C*Bt], -1)

Design:
  - Factorized exponent: P1 = exp(tri + v - 15*qmask) only (u, cmask cancel
    in the row softmax). Column softmax gets them through fm[i] =
    (1-cmask[i])*exp(u[i]): T-GEMM moving operand CF = fm*C and weighted
    column sums c1[j] = sum_i P1[j,i]*fm[i].
  - All matmul operands fp16; psum f32. Host pre-transposes C^T and
    (Q*wm)^T so only the PT1 -> P1 PE transposes remain on-device; their
    psum->sbuf copies carry the row-sum accumulation r for free.
  - Two batches interleaved phase-by-phase so cross-engine latencies hide
    behind the twin batch's PE work; next superbatch's input DMAs are
    issued mid-superbatch to avoid queue head-of-line blocking.
  - Device output is [A, B] fp16; host assembles C and the C*A / C*B
    products in f32 (0.4% of the FLOPs, better precision than fp16).
  - Data-parallel over batch: 32 batches -> 8 cores x 4 batches.
"""
import sys
sys.path.insert(0, "/opt/trn_rl_repo")

import numpy as np
from contextlib import ExitStack

from concourse import bass, bacc, mybir, tile, masks
from concourse.bass import AP
from concourse.bass_utils import run_bass_kernel_spmd

F32 = mybir.dt.float32
F16 = mybir.dt.float16
AF = mybir.ActivationFunctionType
OP = mybir.AluOpType
AX = mybir.AxisListType

B, LC, LQ, D = 32, 1024, 256, 512
NCORES = 8
BPC = B // NCORES          # batches per core
MT, JT, KT = LC // 128, LQ // 128, D // 128   # 8, 2, 4
NEGB = -15.0               # qmask bias in log space; exp(-15) ~ 3e-7 (fp16-safe)

_CACHE = {}


def _build():
    nc = bacc.Bacc("TRN2", target_bir_lowering=False, debug=False)
    C_d = nc.dram_tensor("C16", [BPC, LC, D], F16, kind="ExternalInput")
    CT_d = nc.dram_tensor("CT16", [BPC, D, LC], F16, kind="ExternalInput")
    QWT_d = nc.dram_tensor("QWT16", [BPC, D, LQ], F16, kind="ExternalInput")
    Q_d = nc.dram_tensor("Q16", [BPC, LQ, D], F16, kind="ExternalInput")
    fm_d = nc.dram_tensor("fm16", [BPC, LC], F16, kind="ExternalInput")
    # per-partition cols: [BPC, 128, MT+JT] f32; [:, :, :MT]=fm, [:, :, MT:]=biasj
    cols_d = nc.dram_tensor("colsT", [BPC, 128, MT + JT], F32, kind="ExternalInput")
    out_d = nc.dram_tensor("out", [BPC, LC, 2 * D], F16, kind="ExternalOutput")

    with tile.TileContext(nc) as tc, ExitStack() as ctx:
        const = ctx.enter_context(tc.tile_pool(name="const", bufs=1))
        inp = ctx.enter_context(tc.tile_pool(name="inp", bufs=BPC))
        big = ctx.enter_context(tc.tile_pool(name="big", bufs=2))
        mid = ctx.enter_context(tc.tile_pool(name="mid", bufs=2))
        sm = ctx.enter_context(tc.tile_pool(name="sm", bufs=2))
        pst = ctx.enter_context(tc.tile_pool(name="pst", bufs=2, space="PSUM"))
        ptr = ctx.enter_context(tc.tile_pool(name="ptr", bufs=1, space="PSUM"))
        ptt = ctx.enter_context(tc.tile_pool(name="ptt", bufs=2, space="PSUM"))
        pab = ctx.enter_context(tc.tile_pool(name="pab", bufs=3, space="PSUM"))

        ident_f = const.tile([128, 128], F32)
        masks.make_identity(nc, ident_f[:])
        ident = const.tile([128, 128], F16)
        nc.vector.tensor_copy(ident[:], ident_f[:])

        st = {}  # per-batch tile state

        def phase_dma(b):
            s = st.setdefault(b, {})
            # sync queue: QWT (small) then CT per-k, then derive C = CT^T via xbar
            s["QWT"] = inp.tile([128, KT * LQ], F16, tag="QWT", name=f"QWT{b}")
            nc.sync.dma_start(s["QWT"][:].rearrange("p (k j) -> p k j", k=KT),
                              QWT_d.ap()[b].rearrange("(k p) j -> p k j", p=128))
            s["CT"] = inp.tile([128, KT * LC], F16, tag="CT", name=f"CT{b}")
            for k in range(KT):
                nc.sync.dma_start(s["CT"][:, k * LC:(k + 1) * LC],
                                  CT_d.ap()[b, k * 128:(k + 1) * 128, :])
            s["cols"] = inp.tile([128, MT + JT], F32, tag="cols", name=f"cols{b}")
            nc.sync.dma_start(s["cols"][:], cols_d.ap()[b])
            s["Q"] = inp.tile([128, JT * D], F16, tag="Q", name=f"Q{b}")
            nc.sync.dma_start(s["Q"][:].rearrange("p (t d) -> p t d", t=JT),
                              Q_d.ap()[b].rearrange("(t p) d -> p t d", p=128))
            # gpsimd queue: fm broadcast (stride-0) and C16
            s["fmb"] = inp.tile([128, LC], F16, tag="fmb", name=f"fmb{b}")
            src = fm_d.ap()[b]
            nc.gpsimd.dma_start(s["fmb"][:], AP(src.tensor, src.offset, [(0, 128), (1, LC)]))
            s["C"] = big.tile([128, MT * D], F16, tag="C", name=f"C{b % 2}")
            nc.gpsimd.dma_start(s["C"][:].rearrange("p (t d) -> p t d", t=MT),
                                C_d.ap()[b].rearrange("(t p) d -> p t d", p=128))

        def phase_scores(b):
            s = st[b]
            bj = s["cols"][:, MT:MT + JT]
            s["PT1"] = mid.tile([128, JT * LC], F16, tag="PT1", name=f"PT1{b % 2}")
            for jt in range(JT):
                for h in range(2):
                    ps_st = pst.tile([128, 512], F32, tag="pst")
                    for k in range(KT):
                        nc.tensor.matmul(
                            ps_st[:],
                            s["QWT"][:, k * LQ + jt * 128: k * LQ + (jt + 1) * 128],
                            s["CT"][:, k * LC + h * 512: k * LC + (h + 1) * 512],
                            start=(k == 0), stop=(k == KT - 1))
                    nc.scalar.activation(
                        s["PT1"][:, jt * LC + h * 512: jt * LC + (h + 1) * 512],
                        ps_st[:], AF.Exp, bias=bj[:, jt:jt + 1], scale=1.0)
            # CF = fm * C on gpsimd (early, feeds the T phase)
            fmcol = s["cols"][:, 0:MT]
            s["CF"] = big.tile([128, MT * D], F16, tag="CF", name=f"CF{b % 2}")
            for it in range(MT):
                nc.gpsimd.tensor_scalar(
                    s["CF"][:, it * D:(it + 1) * D], s["C"][:, it * D:(it + 1) * D],
                    fmcol[:, it:it + 1], 0.0, OP.mult, OP.add)

        def phase_transpose(b):
            s = st[b]
            # PT1 [j, (it i)] -> P1 [i, (it j)] via PE transposes; the psum->sbuf
            # copies carry the row-sum accumulation r for free
            s["P1"] = mid.tile([128, MT * LQ], F16, tag="P1", name=f"P1{b % 2}")
            s["r_acc"] = sm.tile([128, MT], F32, tag="r_acc", name=f"ra{b % 2}")
            for itp in range(MT // 2):
                ps_tr = ptr.tile([128, 2 * LQ], F16, tag="ptr")
                for sub in range(2):
                    it = itp * 2 + sub
                    for jt in range(JT):
                        nc.tensor.transpose(
                            ps_tr[:, sub * LQ + jt * 128: sub * LQ + (jt + 1) * 128],
                            s["PT1"][:, jt * LC + it * 128: jt * LC + (it + 1) * 128],
                            ident[:])
                    nc.vector.tensor_scalar(
                        s["P1"][:, it * LQ:(it + 1) * LQ],
                        ps_tr[:, sub * LQ:(sub + 1) * LQ], 1.0, 0.0,
                        OP.mult, OP.add, accum_out=s["r_acc"][:, it:it + 1])
            # c1[j] = sum_i PT1[j,i]*fm[i] on vector
            s["c1p"] = sm.tile([128, JT], F32, tag="c1p", name=f"c1p{b % 2}")
            scr = mid.tile([128, LC], F16, tag="scr", name=f"scr{b % 2}")
            for jt in range(JT):
                nc.vector.scalar_tensor_tensor(
                    scr[:], s["PT1"][:, jt * LC:(jt + 1) * LC], 1.0, s["fmb"][:],
                    OP.mult, OP.mult, accum_out=s["c1p"][:, jt:jt + 1])
            s["c1_rec"] = sm.tile([128, JT], F32, tag="c1r", name=f"c1r{b % 2}")
            nc.vector.reciprocal(s["c1_rec"][:], s["c1p"][:])
            s["r_rec"] = sm.tile([128, MT], F32, tag="r_rec", name=f"rr{b % 2}")
            nc.vector.reciprocal(s["r_rec"][:], s["r_acc"][:])

        def phase_T(b):
            s = st[b]
            s["T16"] = sm.tile([128, JT * D], F16, tag="T16", name=f"T16{b % 2}")
            for jt in range(JT):
                ps_t = ptt.tile([128, 512], F32, tag="ptt")
                for it in range(MT):
                    nc.tensor.matmul(
                        ps_t[:],
                        s["P1"][:, it * LQ + jt * 128: it * LQ + (jt + 1) * 128],
                        s["CF"][:, it * D:(it + 1) * D],
                        start=(it == 0), stop=(it == MT - 1))
                nc.vector.tensor_scalar_mul(
                    s["T16"][:, jt * D:(jt + 1) * D], ps_t[:], s["c1_rec"][:, jt:jt + 1])

        def phase_AB(b):
            s = st[b]
            for itp in range(MT // 4):
                o16 = mid.tile([128, 4 * 1024], F16, tag="o16", bufs=3)
                for sub in range(4):
                    it = itp * 4 + sub
                    ps_a = pab.tile([128, 512], F32, tag="pab", name=f"psa{it % 2}")
                    ps_b = pab.tile([128, 512], F32, tag="pab", name=f"psb{it % 2}")
                    for jt in range(JT):
                        lhs = s["PT1"][:, jt * LC + it * 128: jt * LC + (it + 1) * 128]
                        nc.tensor.matmul(ps_a[:], lhs, s["Q"][:, jt * D:(jt + 1) * D],
                                         start=(jt == 0), stop=(jt == JT - 1))
                        nc.tensor.matmul(ps_b[:], lhs, s["T16"][:, jt * D:(jt + 1) * D],
                                         start=(jt == 0), stop=(jt == JT - 1))
                    o = o16[:, sub * 1024:(sub + 1) * 1024]
                    nc.scalar.activation(o[:, 0:512], ps_a[:], AF.Copy,
                                         bias=0.0, scale=s["r_rec"][:, it:it + 1])
                    nc.vector.tensor_scalar_mul(o[:, 512:1024], ps_b[:],
                                                s["r_rec"][:, it:it + 1])
                nc.sync.dma_start(
                    out_d.ap()[b, itp * 512:(itp + 1) * 512, :].rearrange(
                        "(t p) f -> p t f", p=128),
                    o16[:].rearrange("p (t f) -> p t f", t=4))

        # two-batch software pipeline; next superbatch's inputs issued after
        # this superbatch's xbar transposes so they don't block them (FIFO)
        phase_dma(0)
        phase_dma(1)
        for sb in range(BPC // 2):
            b0, b1 = 2 * sb, 2 * sb + 1
            phase_scores(b0)
            phase_scores(b1)
            phase_transpose(b0)
            phase_transpose(b1)
            phase_T(b0)
            phase_T(b1)
            if b1 + 2 < BPC:
                phase_dma(b1 + 1)
                phase_dma(b1 + 2)
            phase_AB(b0)
            phase_AB(b1)
    nc.compile()
    return nc


def _get_nc():
    if "nc" not in _CACHE:
        _CACHE["nc"] = _build()
    return _CACHE["nc"]


def _prep(C, Q, W0, c_mask, q_mask):
    """Host-side precompute: fp16 operands, transposes, bias/scale vectors."""
    f16 = np.float16
    C = np.asarray(C, np.float32)
    Q = np.asarray(Q, np.float32)
    W0 = np.asarray(W0, np.float32)
    cm = np.asarray(c_mask, np.int32)
    qm = np.asarray(q_mask, np.int32)
    wc, wq, wm = W0[:D], W0[D:2 * D], W0[2 * D:]
    u = C @ wc                                     # [B, LC] f32
    v = Q @ wq                                     # [B, LQ] f32
    fm32 = (1.0 - cm).astype(np.float32) * np.exp(u)
    fm16 = fm32.astype(f16)
    biasj = (v + NEGB * qm).astype(np.float32)
    # per-partition col layout: [B, 128, MT+JT]
    colsT = np.concatenate([
        fm32.reshape(B, MT, 128).transpose(0, 2, 1),
        biasj.reshape(B, JT, 128).transpose(0, 2, 1)], axis=2)
    colsT = np.ascontiguousarray(colsT)
    C16 = C.astype(f16)
    CT16 = np.ascontiguousarray(C.transpose(0, 2, 1)).astype(f16)
    QWT16 = np.ascontiguousarray((Q * wm).transpose(0, 2, 1)).astype(f16)
    Q16 = Q.astype(f16)
    return dict(C16=C16, CT16=CT16, QWT16=QWT16, Q16=Q16, fm16=fm16, colsT=colsT)


def kernel(C, Q, W0, c_mask, q_mask):
    nc = _get_nc()
    C = np.ascontiguousarray(np.asarray(C, dtype=np.float32))
    pre = _prep(C, Q, W0, c_mask, q_mask)
    in_maps = []
    for c in range(NCORES):
        s = slice(c * BPC, (c + 1) * BPC)
        in_maps.append({k: np.ascontiguousarray(v[s]) for k, v in pre.items()})
    res = run_bass_kernel_spmd(nc, in_maps, core_ids=list(range(NCORES)))
    out = np.empty((B, LC, 4 * D), np.float32)
    out[:, :, 0:D] = C
    for c in range(NCORES):
        s = slice(c * BPC, (c + 1) * BPC)
        ab = res.results[c]["out"].astype(np.float32)
        A32 = ab[:, :, 0:D]
        B32 = ab[:, :, D:2 * D]
        out[s, :, D:2 * D] = A32
        out[s, :, 2 * D:3 * D] = C[s] * A32
        out[s, :, 3 * D:] = C[s] * B32
    return out


if __name__ == "__main__":
    sys.path.insert(0, "/root/problem")
    import reference
    inputs = {k: np.asarray(v) for k, v in reference.setup_inputs().items()}
    expected = np.asarray(reference.reference(**inputs))
    actual = kernel(**inputs)
    err = np.abs(actual - expected)
    denom = np.abs(expected).max()
    print("max abs err:", err.max(), "rel:", err.max() / denom)


# revision 30
# speedup vs baseline: 1.1506x; 1.0002x over previous
"""Trainium2 Bass kernel for ContextQueryAttention (BiDAF-style), v3.

Math (per batch):
  S[i,j] = u[i] + v[j] + tri[i,j],  tri[i,j] = sum_d C[i,d]*wm[d]*Q[j,d]
  S_row = softmax_j(S + NEG*qmask[j]);  S_col = softmax_i(S + NEG*cmask[i])
  A  = S_row @ Q
  Bt = S_row @ (S_col^T @ C)
  out = concat([C, A, C*A, C*Bt], -1)

Design:
  - Factorized exponent: P1 = exp(tri + v - 15*qmask) only (u, cmask cancel
    in the row softmax). Column softmax gets them through fm[i] =
    (1-cmask[i])*exp(u[i]): T-GEMM moving operand CF = fm*C and weighted
    column sums c1[j] = sum_i P1[j,i]*fm[i].
  - All matmul operands fp16; psum f32. Host pre-transposes C^T and
    (Q*wm)^T so only the PT1 -> P1 PE transposes remain on-device; their
    psum->sbuf copies carry the row-sum accumulation r for free.
  - Two batches interleaved phase-by-phase so cross-engine latencies hide
    behind the twin batch's PE work; next superbatch's input DMAs are
    issued mid-superbatch to avoid queue head-of-line blocking.
  - Device output is [A, B] fp16; host assembles C and the C*A / C*B
    products in f32 (0.4% of the FLOPs, better precision than fp16).
  - Data-parallel over batch: 32 batches -> 8 cores x 4 batches.
"""
import sys
sys.path.insert(0, "/opt/trn_rl_repo")

import numpy as np
from contextlib import ExitStack

from concourse import bass, bacc, mybir, tile, masks
from concourse.bass import AP
from concourse.bass_utils import run_bass_kernel_spmd

F32 = mybir.dt.float32
F16 = mybir.dt.float16
AF = mybir.ActivationFunctionType
OP = mybir.AluOpType
AX = mybir.AxisListType

B, LC, LQ, D = 32, 1024, 256, 512
NCORES = 8
BPC = B // NCORES          # batches per core
MT, JT, KT = LC // 128, LQ // 128, D // 128   # 8, 2, 4
NEGB = -15.0               # qmask bias in log space; exp(-15) ~ 3e-7 (fp16-safe)

_CACHE = {}


def _build():
    nc = bacc.Bacc("TRN2", target_bir_lowering=False, debug=False)
    C_d = nc.dram_tensor("C16", [BPC, LC, D], F16, kind="ExternalInput")
    CT_d = nc.dram_tensor("CT16", [BPC, D, LC], F16, kind="ExternalInput")
    QWT_d = nc.dram_tensor("QWT16", [BPC, D, LQ], F16, kind="ExternalInput")
    Q_d = nc.dram_tensor("Q16", [BPC, LQ, D], F16, kind="ExternalInput")
    fm_d = nc.dram_tensor("fm16", [BPC, LC], F16, kind="ExternalInput")
    # per-partition cols: [BPC, 128, MT+JT] f32; [:, :, :MT]=fm, [:, :, MT:]=biasj
    cols_d = nc.dram_tensor("colsT", [BPC, 128, MT + JT], F32, kind="ExternalInput")
    out_d = nc.dram_tensor("out", [BPC, LC, 2 * D], F16, kind="ExternalOutput")

    with tile.TileContext(nc) as tc, ExitStack() as ctx:
        const = ctx.enter_context(tc.tile_pool(name="const", bufs=1))
        inp = ctx.enter_context(tc.tile_pool(name="inp", bufs=BPC))
        big = ctx.enter_context(tc.tile_pool(name="big", bufs=2))
        mid = ctx.enter_context(tc.tile_pool(name="mid", bufs=2))
        sm = ctx.enter_context(tc.tile_pool(name="sm", bufs=2))
        pst = ctx.enter_context(tc.tile_pool(name="pst", bufs=2, space="PSUM"))
        ptr = ctx.enter_context(tc.tile_pool(name="ptr", bufs=1, space="PSUM"))
        ptt = ctx.enter_context(tc.tile_pool(name="ptt", bufs=2, space="PSUM"))
        pab = ctx.enter_context(tc.tile_pool(name="pab", bufs=3, space="PSUM"))

        ident_f = const.tile([128, 128], F32)
        masks.make_identity(nc, ident_f[:])
        ident = const.tile([128, 128], F16)
        nc.vector.tensor_copy(ident[:], ident_f[:])

        st = {}  # per-batch tile state

        def phase_dma(b):
            s = st.setdefault(b, {})
            # sync queue: QWT (small) then CT per-k, then derive C = CT^T via xbar
            s["QWT"] = inp.tile([128, KT * LQ], F16, tag="QWT", name=f"QWT{b}")
            nc.sync.dma_start(s["QWT"][:].rearrange("p (k j) -> p k j", k=KT),
                              QWT_d.ap()[b].rearrange("(k p) j -> p k j", p=128))
            s["CT"] = inp.tile([128, KT * LC], F16, tag="CT", name=f"CT{b}")
            for k in range(KT):
                nc.sync.dma_start(s["CT"][:, k * LC:(k + 1) * LC],
                                  CT_d.ap()[b, k * 128:(k + 1) * 128, :])
            s["cols"] = inp.tile([128, MT + JT], F32, tag="cols", name=f"cols{b}")
            nc.sync.dma_start(s["cols"][:], cols_d.ap()[b])
            s["Q"] = inp.tile([128, JT * D], F16, tag="Q", name=f"Q{b}")
            nc.sync.dma_start(s["Q"][:].rearrange("p (t d) -> p t d", t=JT),
                              Q_d.ap()[b].rearrange("(t p) d -> p t d", p=128))
            # gpsimd queue: fm broadcast (stride-0) and C16
            s["fmb"] = inp.tile([128, LC], F16, tag="fmb", name=f"fmb{b}")
            src = fm_d.ap()[b]
            nc.gpsimd.dma_start(s["fmb"][:], AP(src.tensor, src.offset, [(0, 128), (1, LC)]))
            s["C"] = big.tile([128, MT * D], F16, tag="C", name=f"C{b % 2}")
            nc.gpsimd.dma_start(s["C"][:].rearrange("p (t d) -> p t d", t=MT),
                                C_d.ap()[b].rearrange("(t p) d -> p t d", p=128))

        def phase_scores(b):
            s = st[b]
            bj = s["cols"][:, MT:MT + JT]
            s["PT1"] = mid.tile([128, JT * LC], F16, tag="PT1", name=f"PT1{b % 2}")
            for jt in range(JT):
                for h in range(2):
                    ps_st = pst.tile([128, 512], F32, tag="pst")
                    for k in range(KT):
                        nc.tensor.matmul(
                            ps_st[:],
                            s["QWT"][:, k * LQ + jt * 128: k * LQ + (jt + 1) * 128],
                            s["CT"][:, k * LC + h * 512: k * LC + (h + 1) * 512],
                            start=(k == 0), stop=(k == KT - 1))
                    nc.scalar.activation(
                        s["PT1"][:, jt * LC + h * 512: jt * LC + (h + 1) * 512],
                        ps_st[:], AF.Exp, bias=bj[:, jt:jt + 1], scale=1.0)
            # CF = fm * C on gpsimd (early, feeds the T phase)
            fmcol = s["cols"][:, 0:MT]
            s["CF"] = big.tile([128, MT * D], F16, tag="CF", name=f"CF{b % 2}")
            for it in range(MT):
                nc.gpsimd.tensor_scalar(
                    s["CF"][:, it * D:(it + 1) * D], s["C"][:, it * D:(it + 1) * D],
                    fmcol[:, it:it + 1], 0.0, OP.mult, OP.add)

        def phase_transpose(b):
            s = st[b]
            # PT1 [j, (it i)] -> P1 [i, (it j)] via PE transposes; the psum->sbuf
            # copies carry the row-sum accumulation r for free
            s["P1"] = mid.tile([128, MT * LQ], F16, tag="P1", name=f"P1{b % 2}")
            s["r_acc"] = sm.tile([128, MT], F32, tag="r_acc", name=f"ra{b % 2}")
            for itp in range(MT // 2):
                ps_tr = ptr.tile([128, 2 * LQ], F16, tag="ptr")
                for sub in range(2):
                    it = itp * 2 + sub
                    for jt in range(JT):
                        nc.tensor.transpose(
                            ps_tr[:, sub * LQ + jt * 128: sub * LQ + (jt + 1) * 128],
                            s["PT1"][:, jt * LC + it * 128: jt * LC + (it + 1) * 128],
                            ident[:])
                    nc.vector.tensor_scalar(
                        s["P1"][:, it * LQ:(it + 1) * LQ],
                        ps_tr[:, sub * LQ:(sub + 1) * LQ], 1.0, 0.0,
                        OP.mult, OP.add, accum_out=s["r_acc"][:, it:it + 1])
            # c1[j] = sum_i PT1[j,i]*fm[i] on vector
            s["c1p"] = sm.tile([128, JT], F32, tag="c1p", name=f"c1p{b % 2}")
            scr = mid.tile([128, LC], F16, tag="scr", name=f"scr{b % 2}")
            for jt in range(JT):
                nc.vector.scalar_tensor_tensor(
                    scr[:], s["PT1"][:, jt * LC:(jt + 1) * LC], 1.0, s["fmb"][:],
                    OP.mult, OP.mult, accum_out=s["c1p"][:, jt:jt + 1])
            s["c1_rec"] = sm.tile([128, JT], F32, tag="c1r", name=f"c1r{b % 2}")
            nc.vector.reciprocal(s["c1_rec"][:], s["c1p"][:])
            s["r_rec"] = sm.tile([128, MT], F32, tag="r_rec", name=f"rr{b % 2}")
            nc.vector.reciprocal(s["r_rec"][:], s["r_acc"][:])

        def phase_T(b):
            s = st[b]
            s["T16"] = sm.tile([128, JT * D], F16, tag="T16", name=f"T16{b % 2}")
            for jt in range(JT):
                ps_t = ptt.tile([128, 512], F32, tag="ptt")
                for it in range(MT):
                    nc.tensor.matmul(
                        ps_t[:],
                        s["P1"][:, it * LQ + jt * 128: it * LQ + (jt + 1) * 128],
                        s["CF"][:, it * D:(it + 1) * D],
                        start=(it == 0), stop=(it == MT - 1))
                nc.vector.tensor_scalar_mul(
                    s["T16"][:, jt * D:(jt + 1) * D], ps_t[:], s["c1_rec"][:, jt:jt + 1])

        def phase_AB(b):
            s = st[b]
            for itp in range(MT // 4):
                o16 = mid.tile([128, 4 * 1024], F16, tag="o16", bufs=3)
                for sub in range(4):
                    it = itp * 4 + sub
                    ps_a = pab.tile([128, 512], F32, tag="pab", name=f"psa{it % 2}")
                    ps_b = pab.tile([128, 512], F32, tag="pab", name=f"psb{it % 2}")
                    for jt in range(JT):
                        lhs = s["PT1"][:, jt * LC + it * 128: jt * LC + (it + 1) * 128]
                        nc.tensor.matmul(ps_a[:], lhs, s["Q"][:, jt * D:(jt + 1) * D],
                                         start=(jt == 0), stop=(jt == JT - 1))
                        nc.tensor.matmul(ps_b[:], lhs, s["T16"][:, jt * D:(jt + 1) * D],
                                         start=(jt == 0), stop=(jt == JT - 1))
                    o = o16[:, sub * 1024:(sub + 1) * 1024]
                    nc.scalar.activation(o[:, 0:512], ps_a[:], AF.Copy,
                                         bias=0.0, scale=s["r_rec"][:, it:it + 1])
                    nc.vector.tensor_scalar_mul(o[:, 512:1024], ps_b[:],
                                                s["r_rec"][:, it:it + 1])
                nc.sync.dma_start(
                    out_d.ap()[b, itp * 512:(itp + 1) * 512, :].rearrange(
                        "(t p) f -> p t f", p=128),
                    o16[:].rearrange("p (t f) -> p t f", t=4))

        # two-batch software pipeline; next superbatch's inputs issued after
        # this superbatch's xbar transposes so they don't block them (FIFO)
        phase_dma(0)
        phase_dma(1)
        for sb in range(BPC // 2):
            b0, b1 = 2 * sb, 2 * sb + 1
            phase_scores(b0)
            phase_scores(b1)
            phase_transpose(b0)
            phase_transpose(b1)
            phase_T(b0)
            phase_T(b1)
            if b1 + 2 < BPC:
                phase_dma(b1 + 1)
                phase_dma(b1 + 2)
            phase_AB(b0)
            phase_AB(b1)
    nc.compile()
    return nc


def _get_nc():
    if "nc" not in _CACHE:
        _CACHE["nc"] = _build()
    return _CACHE["nc"]


def _prep(C, Q, W0, c_mask, q_mask):
    """Host-side precompute: fp16 operands, transposes, bias/scale vectors."""
    f16 = np.float16
    C = np.asarray(C, np.float32)
    Q = np.asarray(Q, np.float32)
    W0 = np.asarray(W0, np.float32)
    cm = np.asarray(c_mask, np.int32)
    qm = np.asarray(q_mask, np.int32)
    wc, wq, wm = W0[:D], W0[D:2 * D], W0[2 * D:]
    u = C @ wc                                     # [B, LC] f32
    v = Q @ wq                                     # [B, LQ] f32
    fm32 = (1.0 - cm).astype(np.float32) * np.exp(u)
    fm16 = fm32.astype(f16)
    biasj = (v + NEGB * qm).astype(np.float32)
    # per-partition col layout: [B, 128, MT+JT]
    colsT = np.concatenate([
        fm32.reshape(B, MT, 128).transpose(0, 2, 1),
        biasj.reshape(B, JT, 128).transpose(0, 2, 1)], axis=2)
    colsT = np.ascontiguousarray(colsT)
    C16 = C.astype(f16)
    CT16 = np.ascontiguousarray(C.transpose(0, 2, 1)).astype(f16)
    QWT16 = np.ascontiguousarray((Q * wm).transpose(0, 2, 1)).astype(f16)
    Q16 = Q.astype(f16)
    return dict(C16=C16, CT16=CT16, QWT16=QWT16, Q16=Q16, fm16=fm16, colsT=colsT)


def kernel(C, Q, W0, c_mask, q_mask):
    nc = _get_nc()
    C = np.ascontiguousarray(np.asarray(C, dtype=np.float32))
    pre = _prep(C, Q, W0, c_mask, q_mask)
    in_maps = []
    for c in range(NCORES):
        s = slice(c * BPC, (c + 1) * BPC)
        in_maps.append({k: np.ascontiguousarray(v[s]) for k, v in pre.items()})
    res = run_bass_kernel_spmd(nc, in_maps, core_ids=list(range(NCORES)))
    out = np.empty((B, LC, 4 * D), np.float32)
    out[:, :, 0:D] = C
    for c in range(NCORES):
        s = slice(c * BPC, (c + 1) * BPC)
        ab = res.results[c]["out"].astype(np.float32)
        A32 = ab[:, :, 0:D]
        B32 = ab[:, :, D:2 * D]
        out[s, :, D:2 * D] = A32
        out[s, :, 2 * D:3 * D] = C[s] * A32
        out[s, :, 3 * D:] = C[s] * B32
    return out


if __name__ == "__main__":
    sys.path.insert(0, "/root/problem")
    import reference
    inputs = {k: np.asarray(v) for k, v in reference.setup_inputs().items()}
    expected = np.asarray(reference.reference(**inputs))
    actual = kernel(**inputs)
    err = np.abs(actual - expected)
    denom = np.abs(expected).max()
    print("max abs err:", err.max(), "rel:", err.max() / denom)
